# revision 2
# baseline (speedup 1.0000x reference)
"""GAT base layer on 8 TRN2 NeuronCores (Bass/Tile, SPMD).

out[n] = (sum_{e: s_e=n} w_e * h[t_e]) / (sum w_e),  h = x@W.T + b,
w_e = exp(leaky_relu(e_src[s_e] + e_dst[t_e])).

Linearity: aggregate raw x rows, apply W once per node after the
reduction:  out = (W @ aggx) / div + b.

Sharding: edges sorted by source; core c owns source nodes
[c*12500, (c+1)*12500).  No inter-core communication.

Per core the edge stream is packed into blocks of <=128 source slots and
<=2048 edges (16 tiles of 128 edges).  Per 128-edge tile:
 - x rows gathered by t via one indirect DMA (128 rows, one per
   partition — the only offset layout the SWDGE runtime supports),
 - e_dst[t_e] computed on-chip: ed = sum_f Xg[e,f] * v_dst[f] via a
   scalar_tensor_tensor with fused accum_out,
 - e_src[s_e] expanded on-chip from the block's contiguous e_src slice:
   accum of (iota == sl) * es_row,
 - the weighted one-hot M_w[e, slot] = w_e * (slot == sl_e) built in one
   tensor_scalar (is_equal then mult),
 - PSUM accumulates agg[f, slot] = Xg.T @ M_w and div[slot] = M_w.T @ 1.
Padding edges carry sl = 128 so their M_w row is all-zero.
"""

import numpy as np

N = 100000
E = 1600000
F = 128
NCORES = 8
NPC = N // NCORES          # source nodes per core
EB = 2048                  # padded edges per block
G = EB // 128              # edge tiles per block
P1T = 512                  # phase-1 column tile
NPAD = 12800               # padded e_src length (25 * P1T)
ALPHA = 0.2


def _host_tables(s, t):
    """Sort edges by source, partition into cores/blocks, build the
    per-block offset/slot tables in the [partition, tile] device layout."""
    order = np.argsort(s, kind="stable")
    ss = s[order].astype(np.int64)
    tt = t[order].astype(np.int64)
    deg = np.bincount(ss, minlength=N)
    assert deg.max() <= EB, "node degree exceeds block capacity"
    node_start = np.concatenate([[0], np.cumsum(deg)])

    blocks = []  # per core: list of (n0, n1, e0, e1)
    for c in range(NCORES):
        blks = []
        n = c * NPC
        n_end = (c + 1) * NPC
        while n < n_end:
            n0 = n
            cnt = 0
            nodes = 0
            while n < n_end and nodes < 128 and cnt + deg[n] <= EB:
                cnt += deg[n]
                n += 1
                nodes += 1
            blks.append((n0, n, int(node_start[n0]), int(node_start[n])))
        blocks.append(blks)
    NB = max(len(b) for b in blocks)

    # Combined per-block table: [0:G] x-row offsets (t), [G:2G] slot-as-f32.
    tbl = np.zeros((NCORES, NB, 128, 2 * G), np.int32)
    slf_pad = np.full((128, G), 128.0, np.float32)
    tbl[:, :, :, G:] = slf_pad.view(np.int32)
    for c in range(NCORES):
        for b, (n0, n1, e0, e1) in enumerate(blocks[c]):
            k = e1 - e0
            te = tt[e0:e1]
            se = ss[e0:e1]
            j = np.arange(k)
            p = j % 128
            g = j // 128
            slf = np.full((128, G), 128.0, np.float32)
            slf[p, g] = (se - n0).astype(np.float32)
            tbl[c, b, p, g] = te
            tbl[c, b, :, G:] = slf.view(np.int32)
    return blocks, NB, tbl


def _build_nc(NB):
    """One SPMD program; all per-core variation comes in via input tables.
    The per-block e_src row is fetched with a tiny indirect DMA (block
    start offsets differ per core, so a compile-time slice cannot work)."""
    import concourse.bass as bass
    import concourse.mybir as mybir
    from concourse.bass import IndirectOffsetOnAxis
    from concourse.tile import TileContext

    f32 = mybir.dt.float32
    i32 = mybir.dt.int32
    Alu = mybir.AluOpType
    Act = mybir.ActivationFunctionType

    nc = bass.Bass()
    xrow = nc.declare_dram_parameter("xrow", [N, F], f32, isOutput=False)
    xTs = nc.declare_dram_parameter("xTs", [F, NPAD], f32, isOutput=False)
    vs = nc.declare_dram_parameter("vs", [F, 1], f32, isOutput=False)
    vdm = nc.declare_dram_parameter("vdm", [128, F], f32, isOutput=False)
    wT = nc.declare_dram_parameter("wT", [F, F], f32, isOutput=False)
    iotam = nc.declare_dram_parameter("iotam", [128, 128], f32, isOutput=False)
    biasm = nc.declare_dram_parameter("biasm", [128, F], f32, isOutput=False)
    ident = nc.declare_dram_parameter("ident", [128, 128], f32, isOutput=False)
    onesc = nc.declare_dram_parameter("onesc", [128, 1], f32, isOutput=False)
    onesr = nc.declare_dram_parameter("onesr", [1, 128], f32, isOutput=False)
    csrc = nc.declare_dram_parameter("csrc", [1, 1], f32, isOutput=False)
    tbl = nc.declare_dram_parameter("tbl", [NB, 128, 2 * G], i32,
                                    isOutput=False)
    esoff = nc.declare_dram_parameter("esoff", [NB, 2, 1], i32,
                                      isOutput=False)
    outb = nc.declare_dram_parameter("outb", [NB, 128, F], f32, isOutput=True)

    es_d = nc.dram_tensor("es_d", [1, NPAD], f32)

    # ---- phase 1: e_src for this core's nodes (from host-sliced xT) ----
    with TileContext(nc) as tc:
        with (
            tc.tile_pool(name="p1c", bufs=1) as p1c,
            tc.tile_pool(name="p1x", bufs=4) as p1x,
            tc.tile_pool(name="p1o", bufs=4) as p1o,
            tc.tile_pool(name="p1p", bufs=2, space="PSUM") as p1p,
        ):
            vs_sb = p1c.tile([F, 1], f32)
            nc.sync.dma_start(out=vs_sb[:, :], in_=vs[:, :])
            cs_sb = p1c.tile([1, 1], f32)
            nc.sync.dma_start(out=cs_sb[:, :], in_=csrc[:, :])
            for i in range(NPAD // P1T):
                xt = p1x.tile([F, P1T], f32)
                nc.sync.dma_start(out=xt[:, :],
                                  in_=xTs[:, i * P1T:(i + 1) * P1T])
                pe = p1p.tile([1, P1T], f32)
                nc.tensor.matmul(pe[:, :], vs_sb[:, :], xt[:, :],
                                 start=True, stop=True)
                ep = p1o.tile([1, P1T], f32)
                nc.scalar.activation(ep[:, :], pe[:, :], Act.Identity,
                                     bias=cs_sb[:, :], scale=1.0)
                nc.sync.dma_start(out=es_d[:, i * P1T:(i + 1) * P1T],
                                  in_=ep[:, :])

    # ---- phase 2 ----
    with TileContext(nc) as tc:
        with (
            tc.tile_pool(name="cst", bufs=1) as cst,
            tc.tile_pool(name="tblp", bufs=4) as tblp,
            tc.tile_pool(name="xg", bufs=4) as xgp,
            tc.tile_pool(name="sml", bufs=4) as sml,
            tc.tile_pool(name="mw", bufs=4) as mwp,
            tc.tile_pool(name="scr", bufs=4) as scrp,
            tc.tile_pool(name="fin", bufs=3) as finp,
            tc.tile_pool(name="outp", bufs=3) as outp,
            tc.tile_pool(name="pag", bufs=2, space="PSUM") as pag,
            tc.tile_pool(name="pdv", bufs=2, space="PSUM") as pdv,
            tc.tile_pool(name="pfi", bufs=1, space="PSUM") as pfi,
            tc.tile_pool(name="ptr", bufs=1, space="PSUM") as ptr,
        ):
            iota_sb = cst.tile([128, 128], f32)
            nc.sync.dma_start(out=iota_sb[:, :], in_=iotam[:, :])
            vd_sb = cst.tile([128, F], f32)
            nc.sync.dma_start(out=vd_sb[:, :], in_=vdm[:, :])
            wT_sb = cst.tile([F, F], f32)
            nc.sync.dma_start(out=wT_sb[:, :], in_=wT[:, :])
            bias_sb = cst.tile([128, F], f32)
            nc.sync.dma_start(out=bias_sb[:, :], in_=biasm[:, :])
            id_sb = cst.tile([128, 128], f32)
            nc.sync.dma_start(out=id_sb[:, :], in_=ident[:, :])
            ones_sb = cst.tile([128, 1], f32)
            nc.sync.dma_start(out=ones_sb[:, :], in_=onesc[:, :])
            onesr_sb = cst.tile([1, 128], f32)
            nc.sync.dma_start(out=onesr_sb[:, :], in_=onesr[:, :])


            for b in range(NB):
                tb = tblp.tile([128, 2 * G], i32)
                nc.sync.dma_start(out=tb[:, :], in_=tbl[b, :, :])
                sf = tb[:, G:2 * G].bitcast(f32)
                eo = tblp.tile([2, 1], i32)
                nc.sync.dma_start(out=eo[:, :], in_=esoff[b, :, :])
                esl = sml.tile([2, 128], f32)
                nc.gpsimd.indirect_dma_start(
                    esl[:, :], None, es_d[:, :],
                    IndirectOffsetOnAxis(ap=eo[:, :], axis=1))
                peb = pfi.tile([128, 128], f32)
                nc.tensor.matmul(peb[:, :], onesr_sb[:, :], esl[0:1, :],
                                 start=True, stop=True)
                es_bc = scrp.tile([128, 128], f32)
                nc.scalar.activation(es_bc[:, :], peb[:, :], Act.Copy)

                Xg = xgp.tile([128, G, F], f32)
                edc = sml.tile([128, G], f32)
                esc = sml.tile([128, G], f32)
                for g in range(G):
                    nc.gpsimd.indirect_dma_start(
                        Xg[:, g, :], None, xrow[:, :],
                        IndirectOffsetOnAxis(ap=tb[:, g:g + 1], axis=0))
                    # ed[e] = sum_f Xg[e,f] * v_dst[f]
                    s1 = scrp.tile([128, F], f32)
                    nc.vector.scalar_tensor_tensor(
                        s1[:, :], Xg[:, g, :], 1.0, vd_sb[:, :],
                        Alu.bypass, Alu.mult, accum_out=edc[:, g:g + 1])
                    # es[e] = sum_slot (iota==sl_e) * es_row[slot]
                    s2 = scrp.tile([128, 128], f32)
                    nc.vector.scalar_tensor_tensor(
                        s2[:, :], iota_sb[:, :], sf[:, g:g + 1], es_bc[:, :],
                        Alu.is_equal, Alu.mult, accum_out=esc[:, g:g + 1])

                lg = sml.tile([128, G], f32)
                nc.vector.tensor_tensor(lg[:, :], edc[:, :], esc[:, :],
                                        Alu.add)
                lr = sml.tile([128, G], f32)
                nc.vector.scalar_tensor_tensor(lr[:, :], lg[:, :], ALPHA,
                                               lg[:, :], Alu.mult, Alu.max)
                wv = sml.tile([128, G], f32)
                nc.scalar.activation(wv[:, :], lr[:, :], Act.Exp)

                pa = pag.tile([128, 128], f32)
                pd = pdv.tile([128, 1], f32)
                for g in range(G):
                    Mw = mwp.tile([128, 128], f32)
                    nc.vector.tensor_scalar(Mw[:, :], iota_sb[:, :],
                                            sf[:, g:g + 1], wv[:, g:g + 1],
                                            Alu.is_equal, Alu.mult)
                    nc.tensor.matmul(pa[:, :], Xg[:, g, :], Mw[:, :],
                                     start=(g == 0), stop=(g == G - 1))
                    nc.tensor.matmul(pd[:, :], Mw[:, :], ones_sb[:, :],
                                     start=(g == 0), stop=(g == G - 1))

                dcol = sml.tile([128, 1], f32)
                nc.vector.reciprocal(dcol[:, :], pd[:, :])
                agg = finp.tile([128, 128], f32)
                nc.scalar.activation(agg[:, :], pa[:, :], Act.Copy)
                pf = pfi.tile([128, 128], f32)
                nc.tensor.matmul(pf[:, :], wT_sb[:, :], agg[:, :],
                                 start=True, stop=True)
                fo = finp.tile([128, 128], f32)
                nc.scalar.activation(fo[:, :], pf[:, :], Act.Copy)
                pt = ptr.tile([128, 128], f32)
                nc.tensor.transpose(pt[:, :], fo[:, :], id_sb[:, :])
                ob = outp.tile([128, 128], f32)
                nc.vector.scalar_tensor_tensor(ob[:, :], pt[:, :],
                                               dcol[:, :], bias_sb[:, :],
                                               Alu.mult, Alu.add)
                nc.sync.dma_start(out=outb[b, :, :], in_=ob[:, :])
    return nc


def _split_multi_waits(nc, maxw=1):
    """This walrus build rejects instructions carrying more than one sync
    wait; hoist extras onto same-engine NoOps placed directly before."""
    import concourse.mybir as mybir
    for f in nc.m.functions:
        for bb in f.blocks:
            new = []
            for inst in bb.instructions:
                si = inst.sync_info
                waits = list(si.on_wait) if si is not None and si.on_wait else []
                if len(waits) > maxw:
                    keep = waits[-maxw:]
                    extra = waits[:-maxw]
                    for k in range(0, len(extra), maxw):
                        nop = mybir.InstNoOp(
                            name=f"{inst.name}-xw{k}",
                            sync_info=mybir.SyncInfo(
                                on_wait=extra[k:k + maxw], on_update=[]),
                            bass_nofuse=True,
                            engine=inst.engine,
                        )
                        new.append(nop)
                    si.on_wait = keep
                new.append(inst)
            bb.instructions[:] = new


def _apply_tile_drain_patch():
    """Split the tile-exit Drain's many sem waits across sync nops."""
    import concourse.mybir as mybir
    import concourse.tile as tile_mod
    from concourse.vector_clock import ScopedClock

    if getattr(tile_mod.TileContext, "_drain_patch_applied", False):
        return

    def _patched(self, tick_clock, wait_clock):
        nc = self.nc
        collector = nc.sync.nop(nofuse=True)
        wait_clock.add_sem_waits(
            collector.ins, ScopedClock({None: tick_clock.global_clock})
        )
        si = collector.ins.sync_info
        waits = list(si.on_wait) if si is not None and si.on_wait else []
        MAXW = 1
        if len(waits) > MAXW:
            si.on_wait = waits[:MAXW]
            for k in range(MAXW, len(waits), MAXW):
                nop = nc.sync.nop(nofuse=True)
                nop.ins.sync_info = mybir.SyncInfo(
                    on_wait=waits[k:k + MAXW], on_update=[])
        nc.sync.drain()
        nc.all_engine_barrier()
        assert self.sems is not None
        popped = nc._tile_sem_poison_stack.pop()
        assert popped is self._sem_poison
        nc.clear_and_free_semaphores(list(self.sems.allocated().values()))
        nc.all_engine_barrier()

    tile_mod.TileContext._drain_and_barrier = _patched
    tile_mod.TileContext._drain_patch_applied = True


_last_exec_ns = None


def kernel(x, s, t, W, b, a, *, _trace=False):
    import os
    _apply_tile_drain_patch()
    from concourse.bass_utils import run_bass_kernel_spmd

    x = np.ascontiguousarray(x, np.float32)
    s = np.asarray(s, np.int64)
    t = np.asarray(t, np.int64)
    W = np.asarray(W, np.float32)
    b = np.asarray(b, np.float32)
    a = np.asarray(a, np.float32)

    blocks, NB, tbl = _host_tables(s, t)

    # per-(core, block) e_src slice offsets: partition p reads es_d at
    # local index (n0 - c*NPC) + p  (one 4B element per partition)
    esoff = np.zeros((NCORES, NB, 2, 1), np.int32)
    for c in range(NCORES):
        for bi, (n0, n1, _, _) in enumerate(blocks[c]):
            esoff[c, bi, :, 0] = n0 - c * NPC

    nc = _build_nc(NB)
    _split_multi_waits(nc)

    v_src = (W.T @ a[:F]).astype(np.float32)
    v_dst = (W.T @ a[F:]).astype(np.float32)
    c_s = float(b @ a[:F]) + float(b @ a[F:])   # both constants folded in
    xT = np.ascontiguousarray(x.T)
    iota_np = np.arange(128, dtype=np.float32)[None, :]
    id_np = np.eye(128, dtype=np.float32)
    ones_np = np.ones((128, 1), np.float32)
    wT_np = np.ascontiguousarray(W.T)

    in_maps = []
    for c in range(NCORES):
        xTs = np.zeros((F, NPAD), np.float32)
        xTs[:, :NPC] = xT[:, c * NPC:(c + 1) * NPC]
        in_maps.append({
            "xrow": x, "xTs": xTs,
            "vs": v_src[:, None],
            "vdm": np.ascontiguousarray(np.broadcast_to(v_dst, (128, F))),
            "wT": wT_np,
            "iotam": np.ascontiguousarray(np.broadcast_to(iota_np, (128, 128))),
            "biasm": np.ascontiguousarray(np.broadcast_to(b, (128, F))),
            "ident": id_np, "onesc": ones_np,
            "onesr": np.ones((1, 128), np.float32),
            "csrc": np.array([[c_s]], np.float32),
            "tbl": tbl[c], "esoff": esoff[c],
        })

    trace_cores = None
    tc_env = os.environ.get("GAT_TRACE_CORES")
    if tc_env == "all":
        trace_cores = list(range(NCORES))
    elif tc_env:
        trace_cores = [int(v) for v in tc_env.split(",")]
    tmpdir = os.environ.get("GAT_TRACE_DIR") or None
    if tmpdir:
        os.makedirs(tmpdir, exist_ok=True)
    res = run_bass_kernel_spmd(nc, in_maps, list(range(NCORES)),
                               trace=bool(_trace or os.environ.get("GAT_TRACE")),
                               trace_cores=trace_cores, tmpdir=tmpdir)
    global _last_exec_ns
    _last_exec_ns = res.exec_time_ns

    out = np.empty((N, F), np.float32)
    for c in range(NCORES):
        ob = res.results[c]["outb"]
        for bi, (n0, n1, _, _) in enumerate(blocks[c]):
            out[n0:n1] = ob[bi, :n1 - n0, :]
    return out



# revision 3
# speedup vs baseline: 1.0720x; 1.0720x over previous
"""GAT base layer on 8 TRN2 NeuronCores (Bass/Tile, SPMD).

out[n] = (sum_{e: s_e=n} w_e * h[t_e]) / (sum w_e),  h = x@W.T + b,
w_e = exp(leaky_relu(e_src[s_e] + e_dst[t_e])).

Linearity: aggregate raw x rows, apply W once per node after the
reduction:  out = (W @ aggx) / div + b.

Sharding: edges sorted by source; core c owns source nodes
[c*12500, (c+1)*12500).  No inter-core communication.

Per core the edge stream is packed into blocks of <=128 source slots and
<=2048 edges (16 tiles of 128 edges).  Per 128-edge tile:
 - x rows gathered by t via one indirect DMA in bf16 (128 rows, one per
   partition — the only offset layout the SWDGE runtime supports),
 - e_dst[t_e] computed on-chip: ed = sum_f Xg[e,f] * v_dst[f] via a
   scalar_tensor_tensor with fused accum_out,
 - e_src[s_e] expanded on-chip from the block's contiguous e_src slice:
   accum of (iota == sl) * es_row,
 - the weighted one-hot M_w[e, slot] = w_e * (slot == sl_e) built in one
   tensor_scalar (is_equal then mult),
 - PSUM accumulates agg[f, slot] = Xg.T @ M_w and div[slot] = M_w.T @ 1.
Padding edges carry sl = 128 so their M_w row is all-zero.

v1.1: x rows, iota, one-hot and the elementwise chain run in bf16
(tensor + vector 2x); phase 1 writes e_src in block-padded node order so
phase 2 fetches each block's e_src row with a plain 512B DMA instead of
an indirect SWDGE call (the Q7 descriptor-generation engine is the
bottleneck at ~1.1us per indirect DMA).
"""

import numpy as np
import ml_dtypes

N = 100000
E = 1600000
F = 128
NCORES = 8
NPC = N // NCORES          # source nodes per core
EB = 2048                  # padded edges per block
G = EB // 128              # edge tiles per block
P1T = 512                  # phase-1 column tile
ALPHA = 0.2


def _host_tables(s, t):
    """Sort edges by source, partition into cores/blocks, build the
    per-block offset/slot tables in the [partition, tile] device layout."""
    order = np.argsort(s, kind="stable")
    ss = s[order].astype(np.int64)
    tt = t[order].astype(np.int64)
    deg = np.bincount(ss, minlength=N)
    assert deg.max() <= EB, "node degree exceeds block capacity"
    node_start = np.concatenate([[0], np.cumsum(deg)])

    blocks = []  # per core: list of (n0, n1, e0, e1)
    for c in range(NCORES):
        blks = []
        n = c * NPC
        n_end = (c + 1) * NPC
        while n < n_end:
            n0 = n
            cnt = 0
            nodes = 0
            while n < n_end and nodes < 128 and cnt + deg[n] <= EB:
                cnt += deg[n]
                n += 1
                nodes += 1
            blks.append((n0, n, int(node_start[n0]), int(node_start[n])))
        blocks.append(blks)
    NB = max(len(b) for b in blocks)

    # Combined per-block table: [0:G] x-row offsets (t), [G:2G] slot-as-f32.
    tbl = np.zeros((NCORES, NB, 128, 2 * G), np.int32)
    slf_pad = np.full((128, G), 128.0, np.float32)
    tbl[:, :, :, G:] = slf_pad.view(np.int32)
    for c in range(NCORES):
        for b, (n0, n1, e0, e1) in enumerate(blocks[c]):
            k = e1 - e0
            te = tt[e0:e1]
            se = ss[e0:e1]
            j = np.arange(k)
            p = j % 128
            g = j // 128
            slf = np.full((128, G), 128.0, np.float32)
            slf[p, g] = (se - n0).astype(np.float32)
            tbl[c, b, p, g] = te
            tbl[c, b, :, G:] = slf.view(np.int32)
    return blocks, NB, tbl


def _build_nc(NB):
    """One SPMD program; all per-core variation comes in via input tables."""
    import concourse.bass as bass
    import concourse.mybir as mybir
    from concourse.bass import IndirectOffsetOnAxis
    from concourse.tile import TileContext

    f32 = mybir.dt.float32
    bf16 = mybir.dt.bfloat16
    i32 = mybir.dt.int32
    Alu = mybir.AluOpType
    Act = mybir.ActivationFunctionType

    NPAD = -(-(NB * 128) // P1T) * P1T

    nc = bass.Bass()
    xrow = nc.declare_dram_parameter("xrow", [N, F], bf16, isOutput=False)
    xTs = nc.declare_dram_parameter("xTs", [F, NPAD], f32, isOutput=False)
    vs = nc.declare_dram_parameter("vs", [F, 1], f32, isOutput=False)
    vdm = nc.declare_dram_parameter("vdm", [128, F], bf16, isOutput=False)
    wT = nc.declare_dram_parameter("wT", [F, F], f32, isOutput=False)
    iotam = nc.declare_dram_parameter("iotam", [128, 128], bf16, isOutput=False)
    biasm = nc.declare_dram_parameter("biasm", [128, F], f32, isOutput=False)
    ident = nc.declare_dram_parameter("ident", [128, 128], f32, isOutput=False)
    onesc = nc.declare_dram_parameter("onesc", [128, 1], bf16, isOutput=False)
    onesr = nc.declare_dram_parameter("onesr", [1, 128], f32, isOutput=False)
    csrc = nc.declare_dram_parameter("csrc", [1, 1], f32, isOutput=False)
    tbl = nc.declare_dram_parameter("tbl", [NB, 128, 2 * G], i32,
                                    isOutput=False)
    outb = nc.declare_dram_parameter("outb", [NB, 128, F], f32, isOutput=True)

    es_d = nc.dram_tensor("es_d", [1, NPAD], f32)

    # ---- phase 1: e_src for this core's nodes (block-padded order) ----
    with TileContext(nc) as tc:
        with (
            tc.tile_pool(name="p1c", bufs=1) as p1c,
            tc.tile_pool(name="p1x", bufs=4) as p1x,
            tc.tile_pool(name="p1o", bufs=4) as p1o,
            tc.tile_pool(name="p1p", bufs=2, space="PSUM") as p1p,
        ):
            vs_sb = p1c.tile([F, 1], f32)
            nc.sync.dma_start(out=vs_sb[:, :], in_=vs[:, :])
            cs_sb = p1c.tile([1, 1], f32)
            nc.sync.dma_start(out=cs_sb[:, :], in_=csrc[:, :])
            for i in range(NPAD // P1T):
                xt = p1x.tile([F, P1T], f32)
                nc.sync.dma_start(out=xt[:, :],
                                  in_=xTs[:, i * P1T:(i + 1) * P1T])
                pe = p1p.tile([1, P1T], f32)
                nc.tensor.matmul(pe[:, :], vs_sb[:, :], xt[:, :],
                                 start=True, stop=True)
                ep = p1o.tile([1, P1T], f32)
                nc.scalar.activation(ep[:, :], pe[:, :], Act.Identity,
                                     bias=cs_sb[:, :], scale=1.0)
                nc.sync.dma_start(out=es_d[:, i * P1T:(i + 1) * P1T],
                                  in_=ep[:, :])

    # ---- phase 2 ----
    with TileContext(nc) as tc:
        with (
            tc.tile_pool(name="cst", bufs=1) as cst,
            tc.tile_pool(name="tblp", bufs=4) as tblp,
            tc.tile_pool(name="xg", bufs=4) as xgp,
            tc.tile_pool(name="sml", bufs=4) as sml,
            tc.tile_pool(name="mw", bufs=4) as mwp,
            tc.tile_pool(name="scr", bufs=4) as scrp,
            tc.tile_pool(name="fin", bufs=3) as finp,
            tc.tile_pool(name="outp", bufs=3) as outp,
            tc.tile_pool(name="pag", bufs=2, space="PSUM") as pag,
            tc.tile_pool(name="pdv", bufs=2, space="PSUM") as pdv,
            tc.tile_pool(name="pfi", bufs=1, space="PSUM") as pfi,
            tc.tile_pool(name="ptr", bufs=1, space="PSUM") as ptr,
        ):
            iota_sb = cst.tile([128, 128], bf16)
            nc.sync.dma_start(out=iota_sb[:, :], in_=iotam[:, :])
            vd_sb = cst.tile([128, F], bf16)
            nc.sync.dma_start(out=vd_sb[:, :], in_=vdm[:, :])
            wT_sb = cst.tile([F, F], f32)
            nc.sync.dma_start(out=wT_sb[:, :], in_=wT[:, :])
            bias_sb = cst.tile([128, F], f32)
            nc.sync.dma_start(out=bias_sb[:, :], in_=biasm[:, :])
            id_sb = cst.tile([128, 128], f32)
            nc.sync.dma_start(out=id_sb[:, :], in_=ident[:, :])
            ones_sb = cst.tile([128, 1], bf16)
            nc.sync.dma_start(out=ones_sb[:, :], in_=onesc[:, :])
            onesr_sb = cst.tile([1, 128], f32)
            nc.sync.dma_start(out=onesr_sb[:, :], in_=onesr[:, :])

            for b in range(NB):
                tb = tblp.tile([128, 2 * G], i32)
                nc.sync.dma_start(out=tb[:, :], in_=tbl[b, :, :])
                sf = tb[:, G:2 * G].bitcast(f32)
                esr = sml.tile([1, 128], f32)
                nc.sync.dma_start(out=esr[:, :],
                                  in_=es_d[:, b * 128:(b + 1) * 128])
                peb = pfi.tile([128, 128], f32)
                nc.tensor.matmul(peb[:, :], onesr_sb[:, :], esr[:, :],
                                 start=True, stop=True)
                es_bc = scrp.tile([128, 128], bf16)
                nc.scalar.activation(es_bc[:, :], peb[:, :], Act.Copy)

                Xg = xgp.tile([128, G, F], bf16)
                edc = sml.tile([128, G], f32)
                esc = sml.tile([128, G], f32)
                for g in range(G):
                    nc.gpsimd.indirect_dma_start(
                        Xg[:, g, :], None, xrow[:, :],
                        IndirectOffsetOnAxis(ap=tb[:, g:g + 1], axis=0))
                    # ed[e] = sum_f Xg[e,f] * v_dst[f]
                    s1 = scrp.tile([128, F], bf16)
                    nc.vector.scalar_tensor_tensor(
                        s1[:, :], Xg[:, g, :], 1.0, vd_sb[:, :],
                        Alu.bypass, Alu.mult, accum_out=edc[:, g:g + 1])
                    # es[e] = sum_slot (iota==sl_e) * es_row[slot]
                    s2 = scrp.tile([128, 128], bf16)
                    nc.vector.scalar_tensor_tensor(
                        s2[:, :], iota_sb[:, :], sf[:, g:g + 1], es_bc[:, :],
                        Alu.is_equal, Alu.mult, accum_out=esc[:, g:g + 1])

                lg = sml.tile([128, G], f32)
                nc.vector.tensor_tensor(lg[:, :], edc[:, :], esc[:, :],
                                        Alu.add)
                lr = sml.tile([128, G], f32)
                nc.vector.scalar_tensor_tensor(lr[:, :], lg[:, :], ALPHA,
                                               lg[:, :], Alu.mult, Alu.max)
                wv = sml.tile([128, G], f32)
                nc.scalar.activation(wv[:, :], lr[:, :], Act.Exp)

                pa = pag.tile([128, 128], f32)
                pd = pdv.tile([128, 1], f32)
                for g in range(G):
                    Mw = mwp.tile([128, 128], bf16)
                    nc.vector.tensor_scalar(Mw[:, :], iota_sb[:, :],
                                            sf[:, g:g + 1], wv[:, g:g + 1],
                                            Alu.is_equal, Alu.mult)
                    nc.tensor.matmul(pa[:, :], Xg[:, g, :], Mw[:, :],
                                     start=(g == 0), stop=(g == G - 1))
                    nc.tensor.matmul(pd[:, :], Mw[:, :], ones_sb[:, :],
                                     start=(g == 0), stop=(g == G - 1))

                dcol = sml.tile([128, 1], f32)
                nc.vector.reciprocal(dcol[:, :], pd[:, :])
                agg = finp.tile([128, 128], f32)
                nc.scalar.activation(agg[:, :], pa[:, :], Act.Copy)
                pf = pfi.tile([128, 128], f32)
                nc.tensor.matmul(pf[:, :], wT_sb[:, :], agg[:, :],
                                 start=True, stop=True)
                fo = finp.tile([128, 128], f32)
                nc.scalar.activation(fo[:, :], pf[:, :], Act.Copy)
                pt = ptr.tile([128, 128], f32)
                nc.tensor.transpose(pt[:, :], fo[:, :], id_sb[:, :])
                ob = outp.tile([128, 128], f32)
                nc.vector.scalar_tensor_tensor(ob[:, :], pt[:, :],
                                               dcol[:, :], bias_sb[:, :],
                                               Alu.mult, Alu.add)
                nc.sync.dma_start(out=outb[b, :, :], in_=ob[:, :])
    return nc


def _split_multi_waits(nc, maxw=1):
    """This walrus build rejects instructions carrying more than one sync
    wait; hoist extras onto same-engine NoOps placed directly before."""
    import concourse.mybir as mybir
    for f in nc.m.functions:
        for bb in f.blocks:
            new = []
            for inst in bb.instructions:
                si = inst.sync_info
                waits = list(si.on_wait) if si is not None and si.on_wait else []
                if len(waits) > maxw:
                    keep = waits[-maxw:]
                    extra = waits[:-maxw]
                    for k in range(0, len(extra), maxw):
                        nop = mybir.InstNoOp(
                            name=f"{inst.name}-xw{k}",
                            sync_info=mybir.SyncInfo(
                                on_wait=extra[k:k + maxw], on_update=[]),
                            bass_nofuse=True,
                            engine=inst.engine,
                        )
                        new.append(nop)
                    si.on_wait = keep
                new.append(inst)
            bb.instructions[:] = new


def _apply_tile_drain_patch():
    """Split the tile-exit Drain's many sem waits across sync nops."""
    import concourse.mybir as mybir
    import concourse.tile as tile_mod
    from concourse.vector_clock import ScopedClock

    if getattr(tile_mod.TileContext, "_drain_patch_applied", False):
        return

    def _patched(self, tick_clock, wait_clock):
        nc = self.nc
        collector = nc.sync.nop(nofuse=True)
        wait_clock.add_sem_waits(
            collector.ins, ScopedClock({None: tick_clock.global_clock})
        )
        si = collector.ins.sync_info
        waits = list(si.on_wait) if si is not None and si.on_wait else []
        MAXW = 1
        if len(waits) > MAXW:
            si.on_wait = waits[:MAXW]
            for k in range(MAXW, len(waits), MAXW):
                nop = nc.sync.nop(nofuse=True)
                nop.ins.sync_info = mybir.SyncInfo(
                    on_wait=waits[k:k + MAXW], on_update=[])
        nc.sync.drain()
        nc.all_engine_barrier()
        assert self.sems is not None
        popped = nc._tile_sem_poison_stack.pop()
        assert popped is self._sem_poison
        nc.clear_and_free_semaphores(list(self.sems.allocated().values()))
        nc.all_engine_barrier()

    tile_mod.TileContext._drain_and_barrier = _patched
    tile_mod.TileContext._drain_patch_applied = True


_last_exec_ns = None


def kernel(x, s, t, W, b, a, *, _trace=False):
    import os
    _apply_tile_drain_patch()
    from concourse.bass_utils import run_bass_kernel_spmd

    x = np.ascontiguousarray(x, np.float32)
    s = np.asarray(s, np.int64)
    t = np.asarray(t, np.int64)
    W = np.asarray(W, np.float32)
    b = np.asarray(b, np.float32)
    a = np.asarray(a, np.float32)

    blocks, NB, tbl = _host_tables(s, t)
    NPAD = -(-(NB * 128) // P1T) * P1T

    nc = _build_nc(NB)
    _split_multi_waits(nc)

    v_src = (W.T @ a[:F]).astype(np.float32)
    v_dst = (W.T @ a[F:]).astype(np.float32)
    c_s = float(b @ a[:F]) + float(b @ a[F:])   # both constants folded in
    xT = np.ascontiguousarray(x.T)
    x_bf = x.astype(ml_dtypes.bfloat16)
    iota_np = np.arange(128, dtype=np.float32)[None, :]
    id_np = np.eye(128, dtype=np.float32)
    wT_np = np.ascontiguousarray(W.T)

    in_maps = []
    for c in range(NCORES):
        xTs = np.zeros((F, NPAD), np.float32)
        for bi, (n0, n1, _, _) in enumerate(blocks[c]):
            xTs[:, bi * 128:bi * 128 + (n1 - n0)] = xT[:, n0:n1]
        in_maps.append({
            "xrow": x_bf, "xTs": xTs,
            "vs": v_src[:, None],
            "vdm": np.ascontiguousarray(
                np.broadcast_to(v_dst, (128, F))).astype(ml_dtypes.bfloat16),
            "wT": wT_np,
            "iotam": np.ascontiguousarray(
                np.broadcast_to(iota_np, (128, 128))).astype(ml_dtypes.bfloat16),
            "biasm": np.ascontiguousarray(np.broadcast_to(b, (128, F))),
            "ident": id_np,
            "onesc": np.ones((128, 1), np.float32).astype(ml_dtypes.bfloat16),
            "onesr": np.ones((1, 128), np.float32),
            "csrc": np.array([[c_s]], np.float32),
            "tbl": tbl[c],
        })

    trace_cores = None
    tc_env = os.environ.get("GAT_TRACE_CORES")
    if tc_env == "all":
        trace_cores = list(range(NCORES))
    elif tc_env:
        trace_cores = [int(v) for v in tc_env.split(",")]
    tmpdir = os.environ.get("GAT_TRACE_DIR") or None
    if tmpdir:
        os.makedirs(tmpdir, exist_ok=True)
    res = run_bass_kernel_spmd(nc, in_maps, list(range(NCORES)),
                               trace=bool(_trace or os.environ.get("GAT_TRACE")),
                               trace_cores=trace_cores, tmpdir=tmpdir)
    global _last_exec_ns
    _last_exec_ns = res.exec_time_ns

    out = np.empty((N, F), np.float32)
    for c in range(NCORES):
        ob = res.results[c]["outb"]
        for bi, (n0, n1, _, _) in enumerate(blocks[c]):
            out[n0:n1] = ob[bi, :n1 - n0, :]
    return out


# revision 10
# speedup vs baseline: 1.0794x; 1.0069x over previous
"""GAT base layer on 8 TRN2 NeuronCores (Bass/Tile, SPMD).

out[n] = (sum_{e: s_e=n} w_e * h[t_e]) / (sum w_e),  h = x@W.T + b,
w_e = exp(leaky_relu(e_src[s_e] + e_dst[t_e])).

Linearity: aggregate raw x rows, apply W once per node after the
reduction:  out = (W @ aggx) / div + b.

Sharding: edges sorted by source; core c owns source nodes
[c*12500, (c+1)*12500).  No inter-core communication.

Per core the edge stream is packed into blocks of <=128 source slots and
<=2048 edges (16 tiles of 128 edges).  Per 128-edge tile:
 - x rows gathered by t via one indirect DMA in bf16 (128 rows, one per
   partition — the only offset layout the SWDGE runtime supports),
 - e_dst[t_e] computed on-chip: ed = sum_f Xg[e,f] * v_dst[f] via a
   scalar_tensor_tensor with fused accum_out,
 - e_src[s_e] expanded on-chip from the block's contiguous e_src slice:
   accum of (iota == sl) * es_row,
 - the weighted one-hot M_w[e, slot] = w_e * (slot == sl_e) built in one
   tensor_scalar (is_equal then mult),
 - PSUM accumulates agg[f, slot] = Xg.T @ M_w and div[slot] = M_w.T @ 1.
Padding edges carry sl = 128 so their M_w row is all-zero.

v1.1: x rows, iota, one-hot and the elementwise chain run in bf16
(tensor + vector 2x); phase 1 writes e_src in block-padded node order so
phase 2 fetches each block's e_src row with a plain 512B DMA instead of
an indirect SWDGE call (the Q7 descriptor-generation engine is the
bottleneck at ~1.1us per indirect DMA).
"""

import numpy as np
import ml_dtypes

N = 100000
E = 1600000
F = 128
NCORES = 8
NPC = N // NCORES          # source nodes per core
EB = 2048                  # padded edges per block
G = EB // 128              # edge tiles per block
P1T = 512                  # phase-1 column tile
ALPHA = 0.2


def _host_tables(s, t):
    """Sort edges by source, partition into cores/blocks, build the
    per-block offset/slot tables in the [partition, tile] device layout."""
    order = np.argsort(s, kind="stable")
    ss = s[order].astype(np.int64)
    tt = t[order].astype(np.int64)
    deg = np.bincount(ss, minlength=N)
    assert deg.max() <= EB, "node degree exceeds block capacity"
    node_start = np.concatenate([[0], np.cumsum(deg)])

    blocks = []  # per core: list of (n0, n1, e0, e1)
    for c in range(NCORES):
        blks = []
        n = c * NPC
        n_end = (c + 1) * NPC
        while n < n_end:
            n0 = n
            cnt = 0
            nodes = 0
            while n < n_end and nodes < 128 and cnt + deg[n] <= EB:
                cnt += deg[n]
                n += 1
                nodes += 1
            blks.append((n0, n, int(node_start[n0]), int(node_start[n])))
        blocks.append(blks)
    NB = max(len(b) for b in blocks)

    # Combined per-block table: [0:G] x-row offsets (t), [G:2G] slot-as-f32.
    tbl = np.zeros((NCORES, NB, 128, 2 * G), np.int32)
    slf_pad = np.full((128, G), 128.0, np.float32)
    tbl[:, :, :, G:] = slf_pad.view(np.int32)
    for c in range(NCORES):
        for b, (n0, n1, e0, e1) in enumerate(blocks[c]):
            k = e1 - e0
            te = tt[e0:e1]
            se = ss[e0:e1]
            j = np.arange(k)
            p = j % 128
            g = j // 128
            slf = np.full((128, G), 128.0, np.float32)
            slf[p, g] = (se - n0).astype(np.float32)
            tbl[c, b, p, g] = te
            tbl[c, b, :, G:] = slf.view(np.int32)
    return blocks, NB, tbl


def _build_nc(NB):
    """One SPMD program; all per-core variation comes in via input tables."""
    import concourse.bass as bass
    import concourse.mybir as mybir
    from concourse.bass import IndirectOffsetOnAxis
    from concourse.tile import TileContext

    f32 = mybir.dt.float32
    bf16 = mybir.dt.bfloat16
    i32 = mybir.dt.int32
    Alu = mybir.AluOpType
    Act = mybir.ActivationFunctionType

    NPAD = -(-(NB * 128) // P1T) * P1T

    nc = bass.Bass()
    xrow = nc.declare_dram_parameter("xrow", [N, F], bf16, isOutput=False)
    xTs = nc.declare_dram_parameter("xTs", [F, NPAD], f32, isOutput=False)
    vs = nc.declare_dram_parameter("vs", [F, 1], f32, isOutput=False)
    vdm = nc.declare_dram_parameter("vdm", [128, F], bf16, isOutput=False)
    wT = nc.declare_dram_parameter("wT", [F, F], f32, isOutput=False)
    iotam = nc.declare_dram_parameter("iotam", [128, 128], bf16, isOutput=False)
    biasm = nc.declare_dram_parameter("biasm", [128, F], f32, isOutput=False)
    ident = nc.declare_dram_parameter("ident", [128, 128], f32, isOutput=False)
    onesc = nc.declare_dram_parameter("onesc", [128, 1], bf16, isOutput=False)
    onesr = nc.declare_dram_parameter("onesr", [1, 128], f32, isOutput=False)
    csrc = nc.declare_dram_parameter("csrc", [1, 1], f32, isOutput=False)
    tbl = nc.declare_dram_parameter("tbl", [NB, 128, 2 * G], i32,
                                    isOutput=False)
    outb = nc.declare_dram_parameter("outb", [NB, 128, F], f32, isOutput=True)

    # ---- single TileContext: phase 1 (e_src -> SBUF) + phase 2 ----
    # e_src for the core's block-padded nodes lives in a [1, NPAD] SBUF tile
    # on partition 0 (~50KB), written by phase-1 ACT and read directly as the
    # peb-broadcast matmul rhs.  Keeping it in SBUF lets Tile track the
    # phase1->phase2 dependency so both phases share one context and the
    # gather stream starts at t~0, overlapped with phase 1.  Phase-1 xTs
    # loads issue from the scalar-engine HWDGE queue so the Sync queue can
    # serve the block tables immediately.
    with TileContext(nc) as tc:
        with (
            tc.tile_pool(name="cst", bufs=1) as cst,
            tc.tile_pool(name="p1x", bufs=4) as p1x,
            tc.tile_pool(name="tblp", bufs=4) as tblp,
            tc.tile_pool(name="xg", bufs=4) as xgp,
            tc.tile_pool(name="sml", bufs=4) as sml,
            tc.tile_pool(name="mw", bufs=4) as mwp,
            tc.tile_pool(name="scr", bufs=4) as scrp,
            tc.tile_pool(name="fin", bufs=3) as finp,
            tc.tile_pool(name="outp", bufs=3) as outp,
            tc.tile_pool(name="p1p", bufs=1, space="PSUM") as p1p,
            tc.tile_pool(name="pag", bufs=2, space="PSUM") as pag,
            tc.tile_pool(name="pdv", bufs=2, space="PSUM") as pdv,
            tc.tile_pool(name="pfi", bufs=1, space="PSUM") as pfi,
            tc.tile_pool(name="ptr", bufs=1, space="PSUM") as ptr,
        ):
            iota_sb = cst.tile([128, 128], bf16)
            nc.sync.dma_start(out=iota_sb[:, :], in_=iotam[:, :])
            vd_sb = cst.tile([128, F], bf16)
            nc.sync.dma_start(out=vd_sb[:, :], in_=vdm[:, :])
            wT_sb = cst.tile([F, F], f32)
            nc.sync.dma_start(out=wT_sb[:, :], in_=wT[:, :])
            bias_sb = cst.tile([128, F], f32)
            nc.sync.dma_start(out=bias_sb[:, :], in_=biasm[:, :])
            id_sb = cst.tile([128, 128], f32)
            nc.sync.dma_start(out=id_sb[:, :], in_=ident[:, :])
            ones_sb = cst.tile([128, 1], bf16)
            nc.sync.dma_start(out=ones_sb[:, :], in_=onesc[:, :])
            onesr_sb = cst.tile([1, 128], f32)
            nc.sync.dma_start(out=onesr_sb[:, :], in_=onesr[:, :])
            vs_sb = cst.tile([F, 1], f32)
            nc.sync.dma_start(out=vs_sb[:, :], in_=vs[:, :])
            cs_sb = cst.tile([1, 1], f32)
            nc.sync.dma_start(out=cs_sb[:, :], in_=csrc[:, :])
            es_sb = cst.tile([1, NPAD], f32)

            for i in range(NPAD // P1T):
                xt = p1x.tile([F, P1T], f32)
                nc.scalar.dma_start(out=xt[:, :],
                                    in_=xTs[:, i * P1T:(i + 1) * P1T])
                pe = p1p.tile([1, P1T], f32)
                nc.tensor.matmul(pe[:, :], vs_sb[:, :], xt[:, :],
                                 start=True, stop=True)
                nc.scalar.activation(es_sb[:, i * P1T:(i + 1) * P1T], pe[:, :],
                                     Act.Identity, bias=cs_sb[:, :], scale=1.0)

            for b in range(NB):
                tb = tblp.tile([128, 2 * G], i32)
                nc.sync.dma_start(out=tb[:, :], in_=tbl[b, :, :])
                sf = tb[:, G:2 * G].bitcast(f32)
                peb = pfi.tile([128, 128], f32)
                nc.tensor.matmul(peb[:, :], onesr_sb[:, :],
                                 es_sb[:, b * 128:(b + 1) * 128],
                                 start=True, stop=True)
                es_bc = scrp.tile([128, 128], bf16)
                nc.scalar.activation(es_bc[:, :], peb[:, :], Act.Copy)

                Xg = xgp.tile([128, G, F], bf16)
                edc = sml.tile([128, G], f32)
                esc = sml.tile([128, G], f32)
                for g in range(G):
                    nc.gpsimd.indirect_dma_start(
                        Xg[:, g, :], None, xrow[:, :],
                        IndirectOffsetOnAxis(ap=tb[:, g:g + 1], axis=0))
                    # ed[e] = sum_f Xg[e,f] * v_dst[f]
                    s1 = scrp.tile([128, F], bf16)
                    nc.vector.scalar_tensor_tensor(
                        s1[:, :], Xg[:, g, :], 1.0, vd_sb[:, :],
                        Alu.bypass, Alu.mult, accum_out=edc[:, g:g + 1])
                    # es[e] = sum_slot (iota==sl_e) * es_row[slot]
                    s2 = scrp.tile([128, 128], bf16)
                    nc.vector.scalar_tensor_tensor(
                        s2[:, :], iota_sb[:, :], sf[:, g:g + 1], es_bc[:, :],
                        Alu.is_equal, Alu.mult, accum_out=esc[:, g:g + 1])

                lg = sml.tile([128, G], f32)
                nc.vector.tensor_tensor(lg[:, :], edc[:, :], esc[:, :],
                                        Alu.add)
                lr = sml.tile([128, G], f32)
                nc.vector.scalar_tensor_tensor(lr[:, :], lg[:, :], ALPHA,
                                               lg[:, :], Alu.mult, Alu.max)
                wv = sml.tile([128, G], f32)
                nc.scalar.activation(wv[:, :], lr[:, :], Act.Exp)

                pa = pag.tile([128, 128], f32)
                pd = pdv.tile([128, 1], f32)
                for g in range(G):
                    Mw = mwp.tile([128, 128], bf16)
                    nc.vector.tensor_scalar(Mw[:, :], iota_sb[:, :],
                                            sf[:, g:g + 1], wv[:, g:g + 1],
                                            Alu.is_equal, Alu.mult)
                    nc.tensor.matmul(pa[:, :], Xg[:, g, :], Mw[:, :],
                                     start=(g == 0), stop=(g == G - 1))
                    nc.tensor.matmul(pd[:, :], Mw[:, :], ones_sb[:, :],
                                     start=(g == 0), stop=(g == G - 1))

                dcol = sml.tile([128, 1], f32)
                nc.vector.reciprocal(dcol[:, :], pd[:, :])
                agg = finp.tile([128, 128], f32)
                nc.scalar.activation(agg[:, :], pa[:, :], Act.Copy)
                pf = pfi.tile([128, 128], f32)
                nc.tensor.matmul(pf[:, :], wT_sb[:, :], agg[:, :],
                                 start=True, stop=True)
                fo = finp.tile([128, 128], f32)
                nc.scalar.activation(fo[:, :], pf[:, :], Act.Copy)
                pt = ptr.tile([128, 128], f32)
                nc.tensor.transpose(pt[:, :], fo[:, :], id_sb[:, :])
                ob = outp.tile([128, 128], f32)
                nc.vector.scalar_tensor_tensor(ob[:, :], pt[:, :],
                                               dcol[:, :], bias_sb[:, :],
                                               Alu.mult, Alu.add)
                nc.sync.dma_start(out=outb[b, :, :], in_=ob[:, :])
    return nc


def _split_multi_waits(nc, maxw=1):
    """This walrus build rejects instructions carrying more than one sync
    wait; hoist extras onto same-engine NoOps placed directly before."""
    import concourse.mybir as mybir
    for f in nc.m.functions:
        for bb in f.blocks:
            new = []
            for inst in bb.instructions:
                si = inst.sync_info
                waits = list(si.on_wait) if si is not None and si.on_wait else []
                if len(waits) > maxw:
                    keep = waits[-maxw:]
                    extra = waits[:-maxw]
                    for k in range(0, len(extra), maxw):
                        nop = mybir.InstNoOp(
                            name=f"{inst.name}-xw{k}",
                            sync_info=mybir.SyncInfo(
                                on_wait=extra[k:k + maxw], on_update=[]),
                            bass_nofuse=True,
                            engine=inst.engine,
                        )
                        new.append(nop)
                    si.on_wait = keep
                new.append(inst)
            bb.instructions[:] = new


def _apply_tile_drain_patch():
    """Split the tile-exit Drain's many sem waits across sync nops."""
    import concourse.mybir as mybir
    import concourse.tile as tile_mod
    from concourse.vector_clock import ScopedClock

    if getattr(tile_mod.TileContext, "_drain_patch_applied", False):
        return

    def _patched(self, tick_clock, wait_clock):
        nc = self.nc
        collector = nc.sync.nop(nofuse=True)
        wait_clock.add_sem_waits(
            collector.ins, ScopedClock({None: tick_clock.global_clock})
        )
        si = collector.ins.sync_info
        waits = list(si.on_wait) if si is not None and si.on_wait else []
        MAXW = 1
        if len(waits) > MAXW:
            si.on_wait = waits[:MAXW]
            for k in range(MAXW, len(waits), MAXW):
                nop = nc.sync.nop(nofuse=True)
                nop.ins.sync_info = mybir.SyncInfo(
                    on_wait=waits[k:k + MAXW], on_update=[])
        nc.sync.drain()
        nc.all_engine_barrier()
        assert self.sems is not None
        popped = nc._tile_sem_poison_stack.pop()
        assert popped is self._sem_poison
        nc.clear_and_free_semaphores(list(self.sems.allocated().values()))
        nc.all_engine_barrier()

    tile_mod.TileContext._drain_and_barrier = _patched
    tile_mod.TileContext._drain_patch_applied = True


_last_exec_ns = None


def kernel(x, s, t, W, b, a, *, _trace=False):
    import os
    _apply_tile_drain_patch()
    from concourse.bass_utils import run_bass_kernel_spmd

    x = np.ascontiguousarray(x, np.float32)
    s = np.asarray(s, np.int64)
    t = np.asarray(t, np.int64)
    W = np.asarray(W, np.float32)
    b = np.asarray(b, np.float32)
    a = np.asarray(a, np.float32)

    blocks, NB, tbl = _host_tables(s, t)
    NPAD = -(-(NB * 128) // P1T) * P1T

    nc = _build_nc(NB)
    _split_multi_waits(nc)

    v_src = (W.T @ a[:F]).astype(np.float32)
    v_dst = (W.T @ a[F:]).astype(np.float32)
    c_s = float(b @ a[:F]) + float(b @ a[F:])   # both constants folded in
    xT = np.ascontiguousarray(x.T)
    x_bf = x.astype(ml_dtypes.bfloat16)
    iota_np = np.arange(128, dtype=np.float32)[None, :]
    id_np = np.eye(128, dtype=np.float32)
    wT_np = np.ascontiguousarray(W.T)

    in_maps = []
    for c in range(NCORES):
        xTs = np.zeros((F, NPAD), np.float32)
        for bi, (n0, n1, _, _) in enumerate(blocks[c]):
            xTs[:, bi * 128:bi * 128 + (n1 - n0)] = xT[:, n0:n1]
        in_maps.append({
            "xrow": x_bf, "xTs": xTs,
            "vs": v_src[:, None],
            "vdm": np.ascontiguousarray(
                np.broadcast_to(v_dst, (128, F))).astype(ml_dtypes.bfloat16),
            "wT": wT_np,
            "iotam": np.ascontiguousarray(
                np.broadcast_to(iota_np, (128, 128))).astype(ml_dtypes.bfloat16),
            "biasm": np.ascontiguousarray(np.broadcast_to(b, (128, F))),
            "ident": id_np,
            "onesc": np.ones((128, 1), np.float32).astype(ml_dtypes.bfloat16),
            "onesr": np.ones((1, 128), np.float32),
            "csrc": np.array([[c_s]], np.float32),
            "tbl": tbl[c],
        })

    trace_cores = None
    tc_env = os.environ.get("GAT_TRACE_CORES")
    if tc_env == "all":
        trace_cores = list(range(NCORES))
    elif tc_env:
        trace_cores = [int(v) for v in tc_env.split(",")]
    tmpdir = os.environ.get("GAT_TRACE_DIR") or None
    if tmpdir:
        os.makedirs(tmpdir, exist_ok=True)
    res = run_bass_kernel_spmd(nc, in_maps, list(range(NCORES)),
                               trace=bool(_trace or os.environ.get("GAT_TRACE")),
                               trace_cores=trace_cores, tmpdir=tmpdir)
    global _last_exec_ns
    _last_exec_ns = res.exec_time_ns

    out = np.empty((N, F), np.float32)
    for c in range(NCORES):
        ob = res.results[c]["outb"]
        for bi, (n0, n1, _, _) in enumerate(blocks[c]):
            out[n0:n1] = ob[bi, :n1 - n0, :]
    return out


# revision 11
# speedup vs baseline: 1.1069x; 1.0255x over previous
"""GAT base layer on 8 TRN2 NeuronCores (Bass/Tile, SPMD).

out[n] = (sum_{e: s_e=n} w_e * h[t_e]) / (sum w_e),  h = x@W.T + b,
w_e = exp(leaky_relu(e_src[s_e] + e_dst[t_e])).

Linearity: aggregate raw x rows, apply W once per node after the
reduction:  out = (W @ aggx) / div + b.

Sharding: edges sorted by source; core c owns source nodes
[c*12500, (c+1)*12500).  No inter-core communication.

Per core the edge stream is packed into blocks of <=128 source slots and
<=2048 edges (16 tiles of 128 edges).  Per 128-edge tile:
 - x rows gathered by t via one indirect DMA in bf16 (128 rows, one per
   partition — the only offset layout the SWDGE runtime supports),
 - e_dst[t_e] computed on-chip: ed = sum_f Xg[e,f] * v_dst[f] via a
   scalar_tensor_tensor with fused accum_out,
 - e_src[s_e] expanded on-chip from the block's contiguous e_src slice:
   accum of (iota == sl) * es_row,
 - the weighted one-hot M_w[e, slot] = w_e * (slot == sl_e) built in one
   tensor_scalar (is_equal then mult),
 - PSUM accumulates agg[f, slot] = Xg.T @ M_w and div[slot] = M_w.T @ 1.
Padding edges carry sl = 128 so their M_w row is all-zero.

v1.1: x rows, iota, one-hot and the elementwise chain run in bf16
(tensor + vector 2x); phase 1 writes e_src in block-padded node order so
phase 2 fetches each block's e_src row with a plain 512B DMA instead of
an indirect SWDGE call (the Q7 descriptor-generation engine is the
bottleneck at ~1.1us per indirect DMA).
"""

import numpy as np
import ml_dtypes

N = 100000
E = 1600000
F = 128
NCORES = 8
NPC = N // NCORES          # source nodes per core
EB = 1920                  # padded edges per block (G=15: capacity-bound blocks, fewest tiles)
G = EB // 128              # edge tiles per block
P1T = 512                  # phase-1 column tile
ALPHA = 0.2


def _host_tables(s, t):
    """Sort edges by source, partition into cores/blocks, build the
    per-block offset/slot tables in the [partition, tile] device layout."""
    order = np.argsort(s, kind="stable")
    ss = s[order].astype(np.int64)
    tt = t[order].astype(np.int64)
    deg = np.bincount(ss, minlength=N)
    assert deg.max() <= EB, "node degree exceeds block capacity"
    node_start = np.concatenate([[0], np.cumsum(deg)])

    blocks = []  # per core: list of (n0, n1, e0, e1)
    for c in range(NCORES):
        blks = []
        n = c * NPC
        n_end = (c + 1) * NPC
        while n < n_end:
            n0 = n
            cnt = 0
            nodes = 0
            while n < n_end and nodes < 128 and cnt + deg[n] <= EB:
                cnt += deg[n]
                n += 1
                nodes += 1
            blks.append((n0, n, int(node_start[n0]), int(node_start[n])))
        blocks.append(blks)
    NB = max(len(b) for b in blocks)

    # Combined per-block table: [0:G] x-row offsets (t), [G:2G] slot-as-f32.
    tbl = np.zeros((NCORES, NB, 128, 2 * G), np.int32)
    slf_pad = np.full((128, G), 128.0, np.float32)
    tbl[:, :, :, G:] = slf_pad.view(np.int32)
    for c in range(NCORES):
        for b, (n0, n1, e0, e1) in enumerate(blocks[c]):
            k = e1 - e0
            te = tt[e0:e1]
            se = ss[e0:e1]
            j = np.arange(k)
            p = j % 128
            g = j // 128
            slf = np.full((128, G), 128.0, np.float32)
            slf[p, g] = (se - n0).astype(np.float32)
            tbl[c, b, p, g] = te
            tbl[c, b, :, G:] = slf.view(np.int32)
    return blocks, NB, tbl


def _build_nc(NB):
    """One SPMD program; all per-core variation comes in via input tables."""
    import concourse.bass as bass
    import concourse.mybir as mybir
    from concourse.bass import IndirectOffsetOnAxis
    from concourse.tile import TileContext

    f32 = mybir.dt.float32
    bf16 = mybir.dt.bfloat16
    i32 = mybir.dt.int32
    Alu = mybir.AluOpType
    Act = mybir.ActivationFunctionType

    NPAD = -(-(NB * 128) // P1T) * P1T

    nc = bass.Bass()
    xrow = nc.declare_dram_parameter("xrow", [N, F], bf16, isOutput=False)
    xTs = nc.declare_dram_parameter("xTs", [F, NPAD], f32, isOutput=False)
    vs = nc.declare_dram_parameter("vs", [F, 1], f32, isOutput=False)
    vdm = nc.declare_dram_parameter("vdm", [128, F], bf16, isOutput=False)
    wT = nc.declare_dram_parameter("wT", [F, F], f32, isOutput=False)
    iotam = nc.declare_dram_parameter("iotam", [128, 128], bf16, isOutput=False)
    biasm = nc.declare_dram_parameter("biasm", [128, F], f32, isOutput=False)
    ident = nc.declare_dram_parameter("ident", [128, 128], f32, isOutput=False)
    onesc = nc.declare_dram_parameter("onesc", [128, 1], bf16, isOutput=False)
    onesr = nc.declare_dram_parameter("onesr", [1, 128], f32, isOutput=False)
    csrc = nc.declare_dram_parameter("csrc", [1, 1], f32, isOutput=False)
    tbl = nc.declare_dram_parameter("tbl", [NB, 128, 2 * G], i32,
                                    isOutput=False)
    outb = nc.declare_dram_parameter("outb", [NB, 128, F], f32, isOutput=True)

    # ---- single TileContext: phase 1 (e_src -> SBUF) + phase 2 ----
    # e_src for the core's block-padded nodes lives in a [1, NPAD] SBUF tile
    # on partition 0 (~50KB), written by phase-1 ACT and read directly as the
    # peb-broadcast matmul rhs.  Keeping it in SBUF lets Tile track the
    # phase1->phase2 dependency so both phases share one context and the
    # gather stream starts at t~0, overlapped with phase 1.  Phase-1 xTs
    # loads issue from the scalar-engine HWDGE queue so the Sync queue can
    # serve the block tables immediately.
    with TileContext(nc) as tc:
        with (
            tc.tile_pool(name="cst", bufs=1) as cst,
            tc.tile_pool(name="p1x", bufs=4) as p1x,
            tc.tile_pool(name="tblp", bufs=4) as tblp,
            tc.tile_pool(name="xg", bufs=4) as xgp,
            tc.tile_pool(name="sml", bufs=4) as sml,
            tc.tile_pool(name="mw", bufs=4) as mwp,
            tc.tile_pool(name="scr", bufs=4) as scrp,
            tc.tile_pool(name="fin", bufs=3) as finp,
            tc.tile_pool(name="outp", bufs=3) as outp,
            tc.tile_pool(name="p1p", bufs=1, space="PSUM") as p1p,
            tc.tile_pool(name="pag", bufs=2, space="PSUM") as pag,
            tc.tile_pool(name="pdv", bufs=2, space="PSUM") as pdv,
            tc.tile_pool(name="pfi", bufs=1, space="PSUM") as pfi,
            tc.tile_pool(name="ptr", bufs=1, space="PSUM") as ptr,
        ):
            iota_sb = cst.tile([128, 128], bf16)
            nc.sync.dma_start(out=iota_sb[:, :], in_=iotam[:, :])
            vd_sb = cst.tile([128, F], bf16)
            nc.sync.dma_start(out=vd_sb[:, :], in_=vdm[:, :])
            wT_sb = cst.tile([F, F], f32)
            nc.sync.dma_start(out=wT_sb[:, :], in_=wT[:, :])
            bias_sb = cst.tile([128, F], f32)
            nc.sync.dma_start(out=bias_sb[:, :], in_=biasm[:, :])
            id_sb = cst.tile([128, 128], f32)
            nc.sync.dma_start(out=id_sb[:, :], in_=ident[:, :])
            ones_sb = cst.tile([128, 1], bf16)
            nc.sync.dma_start(out=ones_sb[:, :], in_=onesc[:, :])
            onesr_sb = cst.tile([1, 128], f32)
            nc.sync.dma_start(out=onesr_sb[:, :], in_=onesr[:, :])
            vs_sb = cst.tile([F, 1], f32)
            nc.sync.dma_start(out=vs_sb[:, :], in_=vs[:, :])
            cs_sb = cst.tile([1, 1], f32)
            nc.sync.dma_start(out=cs_sb[:, :], in_=csrc[:, :])
            es_sb = cst.tile([1, NPAD], f32)

            for i in range(NPAD // P1T):
                xt = p1x.tile([F, P1T], f32)
                nc.scalar.dma_start(out=xt[:, :],
                                    in_=xTs[:, i * P1T:(i + 1) * P1T])
                pe = p1p.tile([1, P1T], f32)
                nc.tensor.matmul(pe[:, :], vs_sb[:, :], xt[:, :],
                                 start=True, stop=True)
                nc.scalar.activation(es_sb[:, i * P1T:(i + 1) * P1T], pe[:, :],
                                     Act.Identity, bias=cs_sb[:, :], scale=1.0)

            for b in range(NB):
                tb = tblp.tile([128, 2 * G], i32)
                nc.sync.dma_start(out=tb[:, :], in_=tbl[b, :, :])
                sf = tb[:, G:2 * G].bitcast(f32)
                peb = pfi.tile([128, 128], f32)
                nc.tensor.matmul(peb[:, :], onesr_sb[:, :],
                                 es_sb[:, b * 128:(b + 1) * 128],
                                 start=True, stop=True)
                es_bc = scrp.tile([128, 128], bf16)
                nc.scalar.activation(es_bc[:, :], peb[:, :], Act.Copy)

                Xg = xgp.tile([128, G, F], bf16)
                edc = sml.tile([128, G], f32)
                esc = sml.tile([128, G], f32)
                for g in range(G):
                    nc.gpsimd.indirect_dma_start(
                        Xg[:, g, :], None, xrow[:, :],
                        IndirectOffsetOnAxis(ap=tb[:, g:g + 1], axis=0))
                    # ed[e] = sum_f Xg[e,f] * v_dst[f]
                    s1 = scrp.tile([128, F], bf16)
                    nc.vector.scalar_tensor_tensor(
                        s1[:, :], Xg[:, g, :], 1.0, vd_sb[:, :],
                        Alu.bypass, Alu.mult, accum_out=edc[:, g:g + 1])
                    # es[e] = sum_slot (iota==sl_e) * es_row[slot]
                    s2 = scrp.tile([128, 128], bf16)
                    nc.vector.scalar_tensor_tensor(
                        s2[:, :], iota_sb[:, :], sf[:, g:g + 1], es_bc[:, :],
                        Alu.is_equal, Alu.mult, accum_out=esc[:, g:g + 1])

                lg = sml.tile([128, G], f32)
                nc.vector.tensor_tensor(lg[:, :], edc[:, :], esc[:, :],
                                        Alu.add)
                lr = sml.tile([128, G], f32)
                nc.vector.scalar_tensor_tensor(lr[:, :], lg[:, :], ALPHA,
                                               lg[:, :], Alu.mult, Alu.max)
                wv = sml.tile([128, G], f32)
                nc.scalar.activation(wv[:, :], lr[:, :], Act.Exp)

                pa = pag.tile([128, 128], f32)
                pd = pdv.tile([128, 1], f32)
                for g in range(G):
                    Mw = mwp.tile([128, 128], bf16)
                    nc.vector.tensor_scalar(Mw[:, :], iota_sb[:, :],
                                            sf[:, g:g + 1], wv[:, g:g + 1],
                                            Alu.is_equal, Alu.mult)
                    nc.tensor.matmul(pa[:, :], Xg[:, g, :], Mw[:, :],
                                     start=(g == 0), stop=(g == G - 1))
                    nc.tensor.matmul(pd[:, :], Mw[:, :], ones_sb[:, :],
                                     start=(g == 0), stop=(g == G - 1))

                dcol = sml.tile([128, 1], f32)
                nc.vector.reciprocal(dcol[:, :], pd[:, :])
                agg = finp.tile([128, 128], f32)
                nc.scalar.activation(agg[:, :], pa[:, :], Act.Copy)
                pf = pfi.tile([128, 128], f32)
                nc.tensor.matmul(pf[:, :], wT_sb[:, :], agg[:, :],
                                 start=True, stop=True)
                fo = finp.tile([128, 128], f32)
                nc.scalar.activation(fo[:, :], pf[:, :], Act.Copy)
                pt = ptr.tile([128, 128], f32)
                nc.tensor.transpose(pt[:, :], fo[:, :], id_sb[:, :])
                ob = outp.tile([128, 128], f32)
                nc.vector.scalar_tensor_tensor(ob[:, :], pt[:, :],
                                               dcol[:, :], bias_sb[:, :],
                                               Alu.mult, Alu.add)
                nc.sync.dma_start(out=outb[b, :, :], in_=ob[:, :])
    return nc


def _split_multi_waits(nc, maxw=1):
    """This walrus build rejects instructions carrying more than one sync
    wait; hoist extras onto same-engine NoOps placed directly before."""
    import concourse.mybir as mybir
    for f in nc.m.functions:
        for bb in f.blocks:
            new = []
            for inst in bb.instructions:
                si = inst.sync_info
                waits = list(si.on_wait) if si is not None and si.on_wait else []
                if len(waits) > maxw:
                    keep = waits[-maxw:]
                    extra = waits[:-maxw]
                    for k in range(0, len(extra), maxw):
                        nop = mybir.InstNoOp(
                            name=f"{inst.name}-xw{k}",
                            sync_info=mybir.SyncInfo(
                                on_wait=extra[k:k + maxw], on_update=[]),
                            bass_nofuse=True,
                            engine=inst.engine,
                        )
                        new.append(nop)
                    si.on_wait = keep
                new.append(inst)
            bb.instructions[:] = new


def _apply_tile_drain_patch():
    """Split the tile-exit Drain's many sem waits across sync nops."""
    import concourse.mybir as mybir
    import concourse.tile as tile_mod
    from concourse.vector_clock import ScopedClock

    if getattr(tile_mod.TileContext, "_drain_patch_applied", False):
        return

    def _patched(self, tick_clock, wait_clock):
        nc = self.nc
        collector = nc.sync.nop(nofuse=True)
        wait_clock.add_sem_waits(
            collector.ins, ScopedClock({None: tick_clock.global_clock})
        )
        si = collector.ins.sync_info
        waits = list(si.on_wait) if si is not None and si.on_wait else []
        MAXW = 1
        if len(waits) > MAXW:
            si.on_wait = waits[:MAXW]
            for k in range(MAXW, len(waits), MAXW):
                nop = nc.sync.nop(nofuse=True)
                nop.ins.sync_info = mybir.SyncInfo(
                    on_wait=waits[k:k + MAXW], on_update=[])
        nc.sync.drain()
        nc.all_engine_barrier()
        assert self.sems is not None
        popped = nc._tile_sem_poison_stack.pop()
        assert popped is self._sem_poison
        nc.clear_and_free_semaphores(list(self.sems.allocated().values()))
        nc.all_engine_barrier()

    tile_mod.TileContext._drain_and_barrier = _patched
    tile_mod.TileContext._drain_patch_applied = True


_last_exec_ns = None


def kernel(x, s, t, W, b, a, *, _trace=False):
    import os
    _apply_tile_drain_patch()
    from concourse.bass_utils import run_bass_kernel_spmd

    x = np.ascontiguousarray(x, np.float32)
    s = np.asarray(s, np.int64)
    t = np.asarray(t, np.int64)
    W = np.asarray(W, np.float32)
    b = np.asarray(b, np.float32)
    a = np.asarray(a, np.float32)

    blocks, NB, tbl = _host_tables(s, t)
    NPAD = -(-(NB * 128) // P1T) * P1T

    nc = _build_nc(NB)
    _split_multi_waits(nc)

    v_src = (W.T @ a[:F]).astype(np.float32)
    v_dst = (W.T @ a[F:]).astype(np.float32)
    c_s = float(b @ a[:F]) + float(b @ a[F:])   # both constants folded in
    xT = np.ascontiguousarray(x.T)
    x_bf = x.astype(ml_dtypes.bfloat16)
    iota_np = np.arange(128, dtype=np.float32)[None, :]
    id_np = np.eye(128, dtype=np.float32)
    wT_np = np.ascontiguousarray(W.T)

    in_maps = []
    for c in range(NCORES):
        xTs = np.zeros((F, NPAD), np.float32)
        for bi, (n0, n1, _, _) in enumerate(blocks[c]):
            xTs[:, bi * 128:bi * 128 + (n1 - n0)] = xT[:, n0:n1]
        in_maps.append({
            "xrow": x_bf, "xTs": xTs,
            "vs": v_src[:, None],
            "vdm": np.ascontiguousarray(
                np.broadcast_to(v_dst, (128, F))).astype(ml_dtypes.bfloat16),
            "wT": wT_np,
            "iotam": np.ascontiguousarray(
                np.broadcast_to(iota_np, (128, 128))).astype(ml_dtypes.bfloat16),
            "biasm": np.ascontiguousarray(np.broadcast_to(b, (128, F))),
            "ident": id_np,
            "onesc": np.ones((128, 1), np.float32).astype(ml_dtypes.bfloat16),
            "onesr": np.ones((1, 128), np.float32),
            "csrc": np.array([[c_s]], np.float32),
            "tbl": tbl[c],
        })

    trace_cores = None
    tc_env = os.environ.get("GAT_TRACE_CORES")
    if tc_env == "all":
        trace_cores = list(range(NCORES))
    elif tc_env:
        trace_cores = [int(v) for v in tc_env.split(",")]
    tmpdir = os.environ.get("GAT_TRACE_DIR") or None
    if tmpdir:
        os.makedirs(tmpdir, exist_ok=True)
    res = run_bass_kernel_spmd(nc, in_maps, list(range(NCORES)),
                               trace=bool(_trace or os.environ.get("GAT_TRACE")),
                               trace_cores=trace_cores, tmpdir=tmpdir)
    global _last_exec_ns
    _last_exec_ns = res.exec_time_ns

    out = np.empty((N, F), np.float32)
    for c in range(NCORES):
        ob = res.results[c]["outb"]
        for bi, (n0, n1, _, _) in enumerate(blocks[c]):
            out[n0:n1] = ob[bi, :n1 - n0, :]
    return out
